# revision 22
# baseline (speedup 1.0000x reference)
"""Trainium2 Bass kernel for CRF log-likelihood (B=128, S=512, U=1024, T=48).

Strategy (data-parallel, 16 batch rows per core, no collectives):
  - Emission scores H@W on PE (K=1024 in 8 chunks of 128), f16 in / f32 psum,
    in 32 position-tiles of 16 positions (N=256 matmuls) so each emission
    matmul hides inside the chain matmul's pipeline-drain window.
  - Forward+backward scans fused: ONE block-diagonal (113x113) matmul per
    round (fwd states on partitions 0:49, bwd on 64:113 for 64-alignment)
    followed by ONE (113,16) DVE multiply with exp'd scores.  The bwd half
    of H is time-reversed on the host so both chains read the same escan
    column block each round.  256 rounds instead of 511.
  - A 49th "done" state absorbs finished rows (pad logits via a rank-1
    matmul), constant normalizer exp(-C0) per step, corrected on host.
  - Numerator: host recovers score[tag] = log(escan) - (b - C0) from the
    exp'd-scores tensor which is DMA'd out; no extra device work.
"""

import os
from collections import deque

import numpy as np

import concourse.bass as bass
import concourse.tile as tile
from concourse import bacc, mybir
from concourse.bass_utils import run_bass_kernel_spmd

B, S, U, T = 128, 512, 1024, 48
NCORES = 8
NB = B // NCORES          # 16 rows per core
NPOS = NB * S             # 8192 positions per core (pos = p*NB + b)
TA = T + 1                # 49 states (48 tags + "done")
BOT = 64                  # partition base of the bwd chain lane
H113 = BOT + TA           # 113
HALF = S // 2             # 256 positions per direction
NTP = 16                  # positions per emission tile
CPT = NTP * NB            # 256 columns per emission tile
NPAIR = HALF // NTP       # 16 tile pairs
C0 = 4.8                  # per-step log-space normalizer
NEG = -1.0e9
NEGH = -60000.0           # fp16-representable pad logit; exp() == 0
F32 = mybir.dt.float32
BF16 = mybir.dt.bfloat16
F16 = mybir.dt.float16

_PROGRAM = None
LAST_EXEC_NS = None
LAST_RESULT = None


def _build_program():
    nc = bacc.Bacc("TRN2", target_bir_lowering=False, debug=False,
                   enable_asserts=False)

    def din(name, shape, dt=F32):
        return nc.dram_tensor(name, list(shape), dt, kind="ExternalInput").ap()

    def dout(name, shape):
        return nc.dram_tensor(name, list(shape), F32, kind="ExternalOutput").ap()

    # host-pretiled H: [tile, partition, K-chunk*pos*row] — contiguous 4KB
    # per partition per tile so each DMA is 128 big descriptors
    h4 = din("h4", (32, 128, 8 * CPT), F16)
    # w K-chunks (128, 8, 49) + row-0 pad vector [1]*48+[-1] in block 8
    wck = din("wck", (128, 8 * TA + TA), F16)
    mhat = din("mhat", (H113, H113), BF16)   # blockdiag lhsT
    mfin = din("mfin", (H113, H113), BF16)   # final-round lhsT
    # f32 smalls: col0 = [b-C0;NEGb] (rows 0:49) / same (rows 64:113),
    # col1 = [b+start;NEG], cols 2:18 = beta_init (rows 64:113)
    smalls = din("smalls", (H113, 2 + NB))
    padflag = din("padflag", (1, NPOS), F16)  # {0, NEGH}, permuted

    z_out = dout("z_out", (1, NB))
    e_top = dout("e_top", (TA, HALF * NB))
    e_bot = dout("e_bot", (TA, HALF * NB))

    with tile.TileContext(nc) as tc:
        with (
            tc.tile_pool(name="consts", bufs=1) as consts,
            tc.tile_pool(name="hpool", bufs=16) as hpool,
            tc.tile_pool(name="epsum", bufs=4, space="PSUM") as epsum,
            tc.tile_pool(name="psY", bufs=2, space="PSUM") as psY,
            tc.tile_pool(name="psZ", bufs=1, space="PSUM") as psZ,
            tc.tile_pool(name="xpool", bufs=2) as xpool,
        ):
            # ---- critical-path constants first ----
            w_sb = consts.tile([128, 8 * TA + TA], F16, tag="w_sb")
            nc.scalar.dma_start(w_sb[:], wck)
            padf_sb = consts.tile([1, NPOS], F16, tag="padf")
            nc.gpsimd.dma_start(padf_sb[:], padflag)

            hs_tiles = {}

            def dma_tile(t, q, halves=False):
                """t in 0..31: fwd tile j = 2j, bwd tile j = 2j+1."""
                hs = hpool.tile([128, 8 * CPT], F16, tag="hs", name="hs")
                hs_tiles[t] = hs
                # h4 blocks are linear in permuted position: fwd j -> block j,
                # bwd j -> block 16+j
                hb = (t % 2) * NPAIR + t // 2
                if halves:
                    q[0].dma_start(hs[:, 0:4 * CPT], h4[hb, :, 0:4 * CPT])
                    q[1].dma_start(hs[:, 4 * CPT:], h4[hb, :, 4 * CPT:])
                else:
                    q.dma_start(hs[:], h4[hb])

            # first tile pair split in halves across queues
            dma_tile(0, (nc.gpsimd, nc.sync), halves=True)
            dma_tile(1, (nc.gpsimd, nc.scalar), halves=True)

            # ---- remaining constants ----
            mhat_sb = consts.tile([H113, H113], BF16, tag="mhat")
            nc.scalar.dma_start(mhat_sb[:], mhat)
            mfin_sb = consts.tile([H113, H113], BF16, tag="mfin")
            nc.sync.dma_start(mfin_sb[:], mfin)
            onesz_sb = consts.tile([H113, 1], BF16, tag="onesz")
            nc.gpsimd.memset(onesz_sb[BOT:H113, :], 1.0)
            smalls_sb = consts.tile([H113, 2 + NB], F32, tag="smalls")
            nc.scalar.dma_start(smalls_sb[:], smalls)
            bias_e0_sb = smalls_sb[0:TA, 0:1]
            bias_eb_sb = smalls_sb[BOT:H113, 0:1]
            bias_a0_sb = smalls_sb[0:TA, 1:2]
            beta_sb = smalls_sb[:, 2:2 + NB]

            escan2 = consts.tile([H113, HALF * NB], F32, tag="escan2")
            # rows 49:64 must be zero; engine partition bases must be
            # 32-aligned, so clear 32:64 and let the fwd ACTs overwrite 32:49.
            # DVE is idle during the pre-phase.
            nc.vector.memset(escan2[32:BOT, :], 0.0)
            x1 = consts.tile([H113, NB], BF16, tag="x1")
            nc.gpsimd.memset(x1[:], 0.0)

            # remaining h tiles: big lookahead, all on the SWDGE queue
            for t in range(2, 32):
                dma_tile(t, nc.gpsimd)

            def em_ops(t, sp0=0, np_=NTP):
                """Emission ops for positions [sp0, sp0+np_) of tile t."""
                j = t // 2
                is_b = t % 2
                pos0 = ((HALF if is_b else 0) + j * NTP + sp0) * NB
                c0 = (j * NTP + sp0) * NB
                ncol = np_ * NB
                cols = slice(c0, c0 + ncol)
                lo, hi = (BOT, H113) if is_b else (0, TA)
                state = {}
                ops = []

                def mk_mm(hh):
                    def f():
                        if hh == 0:
                            state[0] = epsum.tile([H113, CPT], F32, tag="eps",
                                                  name="eps")
                        ps = state[0]
                        off = hh * CPT + sp0 * NB
                        nc.tensor.matmul(ps[lo:hi, 0:ncol],
                                         w_sb[:, hh * TA:(hh + 1) * TA],
                                         hs_tiles[t][:, off:off + ncol],
                                         start=(hh == 0), stop=False)
                    return f

                def mk_pad():
                    def f():
                        nc.tensor.matmul(state[0][lo:hi, 0:ncol],
                                         w_sb[0:1, 8 * TA:9 * TA],
                                         padf_sb[:, pos0:pos0 + ncol],
                                         start=False, stop=True)
                    return f

                def mk_act():
                    def f():
                        ps = state[0]
                        if is_b:
                            nc.scalar.activation(
                                escan2[BOT:H113, cols], ps[BOT:H113, 0:ncol],
                                mybir.ActivationFunctionType.Exp,
                                bias=bias_eb_sb)
                            if t == 1 and sp0 == 0:
                                nc.vector.tensor_tensor(
                                    x1[BOT:H113, :], beta_sb[BOT:H113, :],
                                    escan2[BOT:H113, 0:NB],
                                    mybir.AluOpType.mult)
                        else:
                            nc.scalar.activation(
                                escan2[0:TA, cols], ps[0:TA, 0:ncol],
                                mybir.ActivationFunctionType.Exp,
                                bias=bias_e0_sb)
                            if t == 0 and sp0 == 0:
                                nc.scalar.activation(
                                    x1[0:TA, :], ps[0:TA, 0:NB],
                                    mybir.ActivationFunctionType.Exp,
                                    bias=bias_a0_sb)
                    return f

                for hh in range(8):
                    ops.append(mk_mm(hh))
                ops.append(mk_pad())
                ops.append(mk_act())
                return ops

            # ---- pre-chain: pair 0 in 8-position subtiles (fast start) ----
            for op_pair in zip(em_ops(0, 0, 8), em_ops(1, 0, 8)):
                for op in op_pair:
                    op()
            for op_pair in zip(em_ops(0, 8, 8), em_ops(1, 8, 8)):
                for op in op_pair:
                    op()

            # emission work for pairs 1..15 paced into chain rounds with
            # sim-time floors (just under the scheduler's chain pace)
            T0_US = 7.0
            R_US = 0.30
            work = {}

            def add_work(r, fn):
                work.setdefault(max(1, min(HALF - 1, r)), []).append(fn)

            for j in range(1, NPAIR):
                base = NTP * (j - 1)
                inter = [op for pair in zip(em_ops(2 * j), em_ops(2 * j + 1))
                         for op in pair]
                for i, op in enumerate(inter):
                    add_work(base + 2 + i * 12 // 20, op)

            # partial escan-out DMAs on the gpsimd queue mid-chain
            add_work(150, lambda: nc.sync.dma_start(
                e_top[:, 0:HALF * NB // 2], escan2[0:TA, 0:HALF * NB // 2]))
            add_work(154, lambda: nc.sync.dma_start(
                e_bot[:, 0:HALF * NB // 2], escan2[BOT:H113, 0:HALF * NB // 2]))

            # ---- the fused chain ----
            x = x1
            for i in range(1, HALF):
                if i in work:
                    floor_ms = (T0_US + R_US * i) * 1e-3
                    with tc.tile_wait_until(floor_ms):
                        for fn in work[i]:
                            fn()
                y = psY.tile([H113, NB], F32, tag="y", name="y")
                nc.tensor.matmul(y[:], mhat_sb, x[:], start=True, stop=True)
                xn = xpool.tile([H113, NB], BF16, tag="x", name="xn")
                nc.vector.tensor_tensor(xn[:], y[:],
                                        escan2[:, i * NB:(i + 1) * NB],
                                        mybir.AluOpType.mult)
                x = xn

            # final round: u_256 at partitions 64:113, dot with rb_256
            y = psY.tile([H113, NB], F32, tag="y", name="y")
            nc.tensor.matmul(y[:], mfin_sb, x[:], start=True, stop=True)
            g = consts.tile([H113, NB], BF16, tag="g")
            nc.vector.tensor_tensor(g[BOT:H113, :], y[BOT:H113, :],
                                    x[BOT:H113, :], mybir.AluOpType.mult)
            zp = psZ.tile([1, NB], F32, tag="zp")
            nc.tensor.matmul(zp[:], onesz_sb[BOT:H113, :], g[BOT:H113, :],
                             start=True, stop=True)
            zsb = consts.tile([1, NB], F32, tag="zsb")
            nc.vector.tensor_copy(zsb[:], zp[:])
            nc.sync.dma_start(z_out, zsb[:])
            nc.scalar.dma_start(e_top[:, HALF * NB // 2:],
                                escan2[0:TA, HALF * NB // 2:])
            nc.gpsimd.dma_start(e_bot[:, HALF * NB // 2:],
                                escan2[BOT:H113, HALF * NB // 2:])

    nc.compile()
    return nc


def _host_inputs(H, W, bb, st, en, tr, tag, s_len, w_mask):
    A = np.exp(tr.astype(np.float64)).astype(np.float32)
    Ahat = np.zeros((TA, TA), np.float32)
    Ahat[:T, :T] = A
    Ahat[:T, T] = np.exp(en).astype(np.float32)
    Ahat[T, T] = 1.0

    import ml_dtypes
    BF = ml_dtypes.bfloat16
    mhat = np.zeros((H113, H113), np.float32)
    mhat[:TA, :TA] = Ahat
    mhat[BOT:, BOT:] = Ahat.T
    mfin = np.zeros((H113, H113), np.float32)
    mfin[:TA, BOT:] = Ahat
    NEGb = np.float32(np.float16(NEGH))


    # w K-chunks (128, 8*49) + pad row vector in block 8
    wck = np.zeros((128, 9 * TA), np.float16)
    Wp = np.zeros((U, TA), np.float16)
    Wp[:, :T] = W.astype(np.float16)
    wck[:, 0:8 * TA] = Wp.reshape(8, 128, TA).transpose(1, 0, 2).reshape(128, 8 * TA)
    wck[0, 8 * TA:8 * TA + T] = 1.0
    wck[0, 8 * TA + T] = -1.0

    bias_e = np.concatenate([(bb - C0).astype(np.float32), [NEGb]])
    bias_a0 = np.concatenate([(bb + st).astype(np.float32), [np.float32(NEG)]])
    smalls = np.zeros((H113, 2 + NB), np.float32)
    smalls[0:TA, 0] = bias_e
    smalls[BOT:H113, 0] = bias_e
    smalls[0:TA, 1] = bias_a0
    smalls[BOT:BOT + T, 2:2 + NB] = np.exp(en).astype(np.float32)[:, None]
    smalls[BOT + T, 2:2 + NB] = 1.0

    perm = np.concatenate([np.arange(HALF), np.arange(S - 1, HALF - 1, -1)])

    shared = {
        "wck": wck,
        "mhat": mhat.astype(BF),
        "mfin": mfin.astype(BF),
        "smalls": smalls,
    }

    s_idx = np.arange(S)
    in_maps = []
    for k in range(NCORES):
        rows = slice(k * NB, (k + 1) * NB)
        len_l = s_len[rows]
        pad = (s_idx[None, :] >= len_l[:, None])          # (NB, S)
        padflag = np.where(pad, NEGb, np.float32(0.0)).T[perm]  # (S, NB)
        im = dict(shared)
        h2l = H[rows][:, perm].transpose(2, 1, 0).astype(np.float16)
        im["h4"] = np.ascontiguousarray(
            h2l.reshape(8, 128, 32, NTP, NB).transpose(2, 1, 0, 3, 4)
            .reshape(32, 128, 8 * CPT))
        im["padflag"] = np.ascontiguousarray(
            padflag.reshape(1, NPOS).astype(np.float16))
        in_maps.append(im)
    return in_maps


def kernel(H, W, b, start_transitions, end_transitions, transitions,
           tag, s_len, w_mask):
    global _PROGRAM
    H = np.asarray(H, np.float32)
    W = np.asarray(W, np.float32)
    bb = np.asarray(b, np.float32)
    st = np.asarray(start_transitions, np.float32)
    en = np.asarray(end_transitions, np.float32)
    tr = np.asarray(transitions, np.float32)
    tag = np.asarray(tag)
    s_len = np.asarray(s_len)
    w_mask = np.asarray(w_mask, np.float32)

    if _PROGRAM is None:
        _PROGRAM = _build_program()
    nc = _PROGRAM

    in_maps = _host_inputs(H, W, bb, st, en, tr, tag, s_len, w_mask)
    trace = bool(int(os.environ.get("KERNEL_TRACE", "0")))
    r = run_bass_kernel_spmd(nc, in_maps, list(range(NCORES)), trace=trace,
                             tmpdir=os.environ.get("KERNEL_TRACE_DIR") or None)
    global LAST_EXEC_NS, LAST_RESULT
    LAST_RESULT = r
    LAST_EXEC_NS = r.exec_time_ns
    res = r.results

    z = np.concatenate([np.asarray(rr["z_out"]).reshape(NB) for rr in res])
    etop = np.stack([np.asarray(rr["e_top"]) for rr in res])  # (NC,TA,HALF*NB)
    ebot = np.stack([np.asarray(rr["e_bot"]) for rr in res])

    # ---- host assembly ----
    perm = np.concatenate([np.arange(HALF), np.arange(S - 1, HALF - 1, -1)])
    logZ = np.log(z.astype(np.float64)) + C0 * (s_len.astype(np.float64) - 1)

    # scores[tag] = log(escan[tag]) - (b[tag] - C0) at unpadded positions
    e_all = np.zeros((NCORES, TA, S, NB), np.float32)
    e_all[:, :, :HALF] = etop.reshape(NCORES, TA, HALF, NB)
    e_all[:, :, perm[HALF:]] = ebot.reshape(NCORES, TA, HALF, NB)
    e_all = e_all.transpose(0, 3, 2, 1).reshape(B, S, TA)  # (B,S,TA)
    ge = np.take_along_axis(e_all, tag[..., None], axis=2)[..., 0]  # (B,S)
    ls = np.where(w_mask > 0, np.log(np.maximum(ge, 1e-30)), 0.0)
    num_emit = (ls.astype(np.float64)
                - (bb[tag].astype(np.float64) - C0) * w_mask).sum(axis=1)

    bidx = np.arange(B)
    num = (st[tag[:, 0]].astype(np.float64)
           + num_emit
           + (bb[tag].astype(np.float64) * w_mask).sum(axis=1)
           + (tr[tag[:, :-1], tag[:, 1:]].astype(np.float64) * w_mask[:, 1:]).sum(axis=1)
           + en[tag[bidx, s_len - 1]].astype(np.float64))
    return (num - logZ).astype(np.float32)
